# revision 20
# baseline (speedup 1.0000x reference)
"""Multi-head attention (B=4, S=2048, E=1024, 16 heads x 64) on 8 Trainium2 cores.

Sharding: core c = 2*b + half handles batch b and heads [8*half, 8*half+8)
(embed slice [512*half, 512*half+512)).  Each core computes its Q/K/V
projections, 8 heads of attention, and a row-parallel out-projection partial
(2048, 1024).  Host unshard: out[b] = partial[2b] + partial[2b+1] + bo.

Per-core device kernel (bf16 matmuls, fp32 accumulation):
  - QT/KT in [d_local, seq] layout (d on partitions) so energy^T = K @ Q^T
    comes out as [k_seq, q_seq] with softmax reductions computable by matmul.
  - softmax without max subtraction (energies are ~N(0,1); exp never overflows)
    with 1/sqrt(64) folded into Wq on the host.
  - exp on the scalar engine straight out of PSUM, bf16 output.
  - V carries an appended ones column so the attn@V matmul (M=65) yields the
    softmax denominator for free in PSUM row 64.
  - normalization: reciprocal_approx_fast of the sums row straight out of
    PSUM, gpsimd partition_broadcast, multiply-on-evict.
  - xq/xk live resident in SBUF (one DMA each); xv is streamed in chunks.
  - scheduling: minimal prologue (QK m=0 + V chunk 0), V chunks 1-3 woven
    into the first attention chunk, QK projections for m+1 woven into m's
    attention, out-projection partially woven into the last m-tile.
"""

import numpy as np
import ml_dtypes

import concourse.bass as bass
import concourse.mybir as mybir
import concourse.tile as tile
import concourse.bacc as bacc
from concourse.bass_utils import run_bass_kernel_spmd

BF16 = mybir.dt.bfloat16
F32 = mybir.dt.float32
NPBF = ml_dtypes.bfloat16

S = 2048          # sequence length
E = 1024          # embed dim
DLOC = 512        # per-core embed slice (8 heads x 64)
HD = 64           # head dim
NHL = 8           # heads per core
KT = E // 128     # 8 contraction tiles for projections
MT = DLOC // 128  # 4 m-tiles of d_local
ST = S // 128     # 16 seq tiles
NCH = S // 512    # 4 seq chunks of 512
EXP = mybir.ActivationFunctionType.Exp


def _build_bass(dump=False):
    nc = bacc.Bacc("TRN2", target_bir_lowering=False, debug=False)

    xqT = nc.dram_tensor("xqT", [E, S], BF16, kind="ExternalInput").ap()
    xkT = nc.dram_tensor("xkT", [E, S], BF16, kind="ExternalInput").ap()
    xvT = nc.dram_tensor("xvT", [E, S], BF16, kind="ExternalInput").ap()
    wq_d = nc.dram_tensor("wq", [E, DLOC], BF16, kind="ExternalInput").ap()
    wk_d = nc.dram_tensor("wk", [E, DLOC], BF16, kind="ExternalInput").ap()
    wv_d = nc.dram_tensor("wv", [E, DLOC], BF16, kind="ExternalInput").ap()
    wo_d = nc.dram_tensor("wo", [DLOC, E], BF16, kind="ExternalInput").ap()
    bq_d = nc.dram_tensor("bq", [128, MT], F32, kind="ExternalInput").ap()
    bk_d = nc.dram_tensor("bk", [128, MT], F32, kind="ExternalInput").ap()
    bv_d = nc.dram_tensor("bv", [1, DLOC], F32, kind="ExternalInput").ap()
    out_d = nc.dram_tensor("out", [S, E], BF16, kind="ExternalOutput").ap()

    xq_r = xqT.rearrange("(kt p) s -> p kt s", p=128)
    xk_r = xkT.rearrange("(kt p) s -> p kt s", p=128)
    xv_r = xvT.rearrange("(kt p) s -> p kt s", p=128)

    with tile.TileContext(nc) as tc:
        _kernel_body(tc, nc, xq_r, xk_r, xv_r, wq_d, wk_d, wv_d, wo_d,
                     bq_d, bk_d, bv_d, out_d, dump=dump)
    nc.compile()
    return nc


def _kernel_body(tc, nc, xq_r, xk_r, xv_r, wq_d, wk_d, wv_d, wo_d,
                 bq_d, bk_d, bv_d, out_d, dump=False):
    from contextlib import ExitStack

    with ExitStack() as ctx:
        wpool = ctx.enter_context(tc.tile_pool(name="weights", bufs=1))
        xpool = ctx.enter_context(tc.tile_pool(name="xstream", bufs=2))
        qkv = ctx.enter_context(tc.tile_pool(name="qkv", bufs=1))
        atp = ctx.enter_context(tc.tile_pool(name="attnt", bufs=4))
        smp = ctx.enter_context(tc.tile_pool(name="small", bufs=2))
        outp = ctx.enter_context(tc.tile_pool(name="outstage", bufs=2))

        # ---- weights / biases / resident inputs to SBUF ----
        wq_sb = wpool.tile([128, KT, DLOC], BF16)
        wk_sb = wpool.tile([128, KT, DLOC], BF16)
        wv_sb = wpool.tile([128, KT, DLOC], BF16)
        wo_sb = wpool.tile([128, MT, E], BF16)
        bq_sb = wpool.tile([128, MT], F32)
        bk_sb = wpool.tile([128, MT], F32)
        bv_row = wpool.tile([1, DLOC], F32)
        bv_bc = wpool.tile([128, DLOC], F32)
        xq_sb = qkv.tile([128, KT, S], BF16)
        xk_sb = qkv.tile([128, KT, S], BF16)

        wq_r = wq_d.rearrange("(kt p) m -> p kt m", p=128)
        wk_r = wk_d.rearrange("(kt p) m -> p kt m", p=128)
        wv_r = wv_d.rearrange("(kt p) m -> p kt m", p=128)

        # DMA issue order == dependency order of the prologue compute:
        # K projections first (attention needs all of K^T), then half of Q,
        # then V's weights and first xv chunks; Q2/Q3 and the remaining xv
        # stream in under the first attention chunk.
        nc.sync.dma_start(wk_sb[:], wk_r)
        nc.sync.dma_start(bk_sb[:], bk_d)
        # trigger the exp activation-table load during the DMA-bound prologue
        warm = wpool.tile([1, 1], F32)
        nc.scalar.activation(warm[:], bk_sb[0:1, 0:1], EXP)
        for nch in range(NCH):
            nc.sync.dma_start(xk_sb[:, :, bass.ts(nch, 512)],
                              xk_r[:, :, bass.ts(nch, 512)])
        nc.sync.dma_start(wq_sb[:], wq_r)
        nc.sync.dma_start(bq_sb[:], bq_d)
        for nch in range(2):
            nc.sync.dma_start(xq_sb[:, :, bass.ts(nch, 512)],
                              xq_r[:, :, bass.ts(nch, 512)])
        nc.sync.dma_start(wv_sb[:], wv_r)
        nc.sync.dma_start(bv_row[:], bv_d)
        xv_tiles = {}

        def xv_dma(hc):
            def fn():
                xt = xpool.tile([128, KT, 256], BF16, tag="xs", name="xv_t")
                nc.sync.dma_start(xt[:], xv_r[:, :, bass.ts(hc, 256)])
                xv_tiles[hc] = xt
            return fn

        xv_dma(0)()
        xv_dma(1)()
        for nch in range(2, NCH):
            nc.sync.dma_start(xq_sb[:, :, bass.ts(nch, 512)],
                              xq_r[:, :, bass.ts(nch, 512)])
        nc.sync.dma_start(wo_sb[:], wo_d.rearrange("(mt p) e -> p mt e", p=128))
        nc.gpsimd.partition_broadcast(bv_bc[:], bv_row[:])

        # ---- persistent per-core tensors ----
        QT_sb = qkv.tile([128, MT, S], BF16)        # [d_loc, seq]
        KT_sb = qkv.tile([128, MT, S], BF16)
        V_sb = qkv.tile([128, ST, NHL, HD + 1], BF16)  # ones col at 64
        oT_sb = qkv.tile([128, MT, S], BF16)        # attn out^T (lhsT of outproj)

        nc.vector.memset(V_sb[:, :, :, HD:HD + 1], 1.0)

        # PSUM: 2x [128,1024] energy/proj slots + 2x [65,1024] attn-out slots.
        pe_pool = ctx.enter_context(tc.tile_pool(name="psum_e", bufs=2, space="PSUM"))
        po_pool = ctx.enter_context(tc.tile_pool(name="psum_o", bufs=2, space="PSUM"))

        def v_proj_group(st):
            xv_t = xv_tiles[st // 2]
            ps = pe_pool.tile([128, 1024], F32, tag="pe", name="ps_v")
            for kt in range(KT):
                nc.tensor.matmul(
                    ps[:, 0:512], xv_t[:, kt, bass.ts(st % 2, 128)],
                    wv_sb[:, kt, :], start=(kt == 0), stop=(kt == KT - 1))
            nc.vector.tensor_tensor(
                V_sb[:, st, :, 0:HD],
                ps[:, 0:512].rearrange("p (h d) -> p h d", d=HD),
                bv_bc.rearrange("p (h d) -> p h d", d=HD),
                mybir.AluOpType.add)

        def qk_proj_group(ti, m, nch):
            x_sb = (xq_sb, xk_sb)[ti]
            w_sb = (wq_sb, wk_sb)[ti]
            b_sb = (bq_sb, bk_sb)[ti]
            dst = (QT_sb, KT_sb)[ti]
            ps = pe_pool.tile([128, 1024], F32, tag="pe", name="ps_qk")
            for kt in range(KT):
                nc.tensor.matmul(
                    ps[:, 0:512], w_sb[:, kt, bass.ts(m, 128)],
                    x_sb[:, kt, bass.ts(nch, 512)],
                    start=(kt == 0), stop=(kt == KT - 1))
            nc.vector.tensor_scalar_add(
                dst[:, m, bass.ts(nch, 512)], ps[:, 0:512], b_sb[:, m:m + 1])

        def outproj_group(qt):
            ob = outp.tile([128, E], BF16, tag="ob")
            for ec in range(2):
                ps = pe_pool.tile([128, 1024], F32, tag="pe", name="ps_o")
                for mm in range(MT):
                    nc.tensor.matmul(
                        ps[:, 0:512], oT_sb[:, mm, bass.ts(qt, 128)],
                        wo_sb[:, mm, bass.ts(ec, 512)],
                        start=(mm == 0), stop=(mm == MT - 1))
                nc.vector.tensor_copy(ob[:, bass.ts(ec, 512)], ps[:, 0:512])
            nc.sync.dma_start(out_d[bass.ts(qt, 128), :], ob[:])

        def attn_chunk(m, hs, q0, weave=None, W=1024):
            """One (head, W-q) attention chunk; weave: {kt: [callables]}.

            attn@V trails the energy/exp pipeline by 2 kt steps so the PE
            never waits on the scalar engine's exp latency.
            """
            h = 2 * m + hs
            psl = slice(64 * hs, 64 * hs + 64)
            nqc = W // 512
            po = po_pool.tile([HD + 1, 1024], F32, tag="po")
            pending = []

            def attn_v(pkt, pat):
                for qc in range(nqc):
                    nc.tensor.matmul(
                        po[:, bass.ts(qc, 512)],
                        V_sb[:, pkt, h, :],
                        pat[:, bass.ts(qc, 512)],
                        start=(pkt == 0), stop=(pkt == ST - 1))

            for kt in range(ST):
                pe = pe_pool.tile([128, 1024], F32, tag="pe")
                for qc in range(nqc):
                    nc.tensor.matmul(
                        pe[:, bass.ts(qc, 512)],
                        KT_sb[psl, m, bass.ts(kt, 128)],
                        QT_sb[psl, m, bass.ds(q0 + qc * 512, 512)],
                        start=True, stop=True)
                at = atp.tile([128, 1024], BF16, tag="at")
                nc.scalar.activation(at[:, 0:W], pe[:, 0:W], EXP)
                pending.append((kt, at))
                if len(pending) > 2:
                    attn_v(*pending.pop(0))
                if weave:
                    for fn in weave.get(kt, ()):
                        fn()
            for pkt, pat in pending:
                attn_v(pkt, pat)

            # ---- normalize + evict ----
            # (broadcast the raw sums, then reciprocal+multiply both on the
            # vector engine: the custom-DVE recip output is only ever read by
            # the same engine, dodging its cross-engine completion hazard)
            s_sb = smp.tile([1, 1024], F32, tag="s")
            nc.vector.tensor_copy(s_sb[0:1, 0:W], po[HD:HD + 1, 0:W])
            bc = smp.tile([HD, 1024], F32, tag="bc")
            nc.gpsimd.partition_broadcast(bc[:, 0:W], s_sb[0:1, 0:W])
            nc.vector.reciprocal_approx_fast(bc[:, 0:W], bc[:, 0:W])
            nc.vector.tensor_tensor(
                oT_sb[64 * hs:64 * hs + HD, m, bass.ds(q0, W)],
                po[0:HD, 0:W], bc[:, 0:W], mybir.AluOpType.mult)

        # ---- prologue: K fully, Q first half ----
        for nch in range(NCH):
            qk_proj_group(1, 0, nch)
        for nch in range(2):
            qk_proj_group(0, 0, nch)

        def v_weave(st):
            return lambda: v_proj_group(st)

        def qk_weave(ti, m, nch):
            return lambda: qk_proj_group(ti, m, nch)

        def op_weave(qt):
            return lambda: outproj_group(qt)

        # ---- attention ----
        # chunk 1 (m0,hs0,qh0): all 16 V groups + Q2/Q3 + xv streaming.
        # V seq-tile st is read by attn@V at iteration st+2 (pending depth
        # 2); weaving st at kt=st keeps PE order correct with margin.
        w00 = {}
        for st in range(ST):
            w00.setdefault(min(st, 14), []).append(v_weave(st))
        for hc in range(2, 8):
            w00.setdefault(2 * hc - 4, []).insert(0, xv_dma(hc))
        w00[5] = w00.get(5, []) + [qk_weave(0, 0, 2)]
        w00[10] = w00.get(10, []) + [qk_weave(0, 0, 3)]
        attn_chunk(0, 0, 0, w00)

        # m -> m+1 projection weave, 2-3 groups per chunk, ordered so that
        # K tiles and the first Q half land well before m+1's first chunk.
        def mw(m, spec):
            return {1 + 5 * g: [qk_weave(ti, m + 1, nch)]
                    for g, (ti, nch) in enumerate(spec)}

        attn_chunk(0, 0, 1024, mw(0, [(0, 0), (0, 1), (1, 0)]))
        attn_chunk(0, 1, 0, mw(0, [(1, 1), (1, 2)]))
        attn_chunk(0, 1, 1024, mw(0, [(1, 3), (0, 2), (0, 3)]))
        for m in (1, 2):
            attn_chunk(m, 0, 0, mw(m, [(0, 0), (0, 1)]))
            attn_chunk(m, 0, 1024, mw(m, [(1, 0), (1, 1)]))
            attn_chunk(m, 1, 0, mw(m, [(1, 2), (1, 3)]))
            attn_chunk(m, 1, 1024, mw(m, [(0, 2), (0, 3)]))
        # m=3: qh-outer so the out-projection weaves into the second half;
        # the last head's window is split in two so qt8-11 also weave in.
        attn_chunk(3, 0, 0)
        attn_chunk(3, 1, 0)
        wo0 = {3 + 2 * g: [op_weave(qt)] for g, qt in enumerate(range(6))}
        attn_chunk(3, 0, 1024, wo0)
        attn_chunk(3, 1, 1024, {3: [op_weave(6)], 8: [op_weave(7)]}, W=512)
        attn_chunk(3, 1, 1536,
                   {4 + 3 * g: [op_weave(qt)] for g, qt in enumerate(range(8, 12))},
                   W=512)

        if dump:
            d_qt = nc.dram_tensor("d_qt", [128, MT, S], BF16, kind="ExternalOutput").ap()
            d_kt = nc.dram_tensor("d_kt", [128, MT, S], BF16, kind="ExternalOutput").ap()
            d_v = nc.dram_tensor("d_v", [128, ST, NHL, HD + 1], BF16, kind="ExternalOutput").ap()
            d_ot = nc.dram_tensor("d_ot", [128, MT, S], BF16, kind="ExternalOutput").ap()
            nc.sync.dma_start(d_qt, QT_sb[:])
            nc.sync.dma_start(d_kt, KT_sb[:])
            nc.sync.dma_start(d_v, V_sb[:])
            nc.sync.dma_start(d_ot, oT_sb[:])

        # ---- out-projection tail ----
        for qt in range(12, ST):
            outproj_group(qt)


_CACHED = {}


def _get_bass():
    if "nc" not in _CACHED:
        _CACHED["nc"] = _build_bass()
    return _CACHED["nc"]


def _prep_core_inputs(c, query, key, value, Wq, bq, Wk, bk, Wv, bv, Wo):
    b, half = c // 2, c % 2
    sl = slice(DLOC * half, DLOC * half + DLOC)
    bq_sl = (bq[sl] * 0.125).astype(np.float32).reshape(MT, 128).T.copy()
    bk_sl = bk[sl].astype(np.float32).reshape(MT, 128).T.copy()
    return {
        "xqT": np.ascontiguousarray(query[b].T).astype(NPBF),
        "xkT": np.ascontiguousarray(key[b].T).astype(NPBF),
        "xvT": np.ascontiguousarray(value[b].T).astype(NPBF),
        "wq": np.ascontiguousarray(Wq[sl, :].T * 0.125).astype(NPBF),
        "wk": np.ascontiguousarray(Wk[sl, :].T).astype(NPBF),
        "wv": np.ascontiguousarray(Wv[sl, :].T).astype(NPBF),
        "wo": np.ascontiguousarray(Wo[:, sl].T).astype(NPBF),
        "bq": np.ascontiguousarray(bq_sl),
        "bk": np.ascontiguousarray(bk_sl),
        "bv": bv[sl].astype(np.float32).reshape(1, DLOC).copy(),
    }


def kernel(query, key, value, Wq, bq, Wk, bk, Wv, bv, Wo, bo,
           trace=False, **run_kwargs):
    query = np.asarray(query, np.float32)
    key = np.asarray(key, np.float32)
    value = np.asarray(value, np.float32)
    Wq, Wk, Wv, Wo = (np.asarray(w, np.float32) for w in (Wq, Wk, Wv, Wo))
    bq, bk, bv, bo = (np.asarray(x, np.float32) for x in (bq, bk, bv, bo))

    nc = _get_bass()
    in_maps = [_prep_core_inputs(c, query, key, value, Wq, bq, Wk, bk, Wv, bv, Wo)
               for c in range(8)]
    res = run_bass_kernel_spmd(nc, in_maps, core_ids=list(range(8)),
                               trace=trace, **run_kwargs)
    _CACHED["last_result"] = res

    B = query.shape[0]
    out = np.empty((B, S, E), np.float32)
    for b in range(B):
        out[b] = (res.results[2 * b]["out"].astype(np.float32)
                  + res.results[2 * b + 1]["out"].astype(np.float32) + bo)
    return out


# revision 21
# speedup vs baseline: 1.0057x; 1.0057x over previous
"""Multi-head attention (B=4, S=2048, E=1024, 16 heads x 64) on 8 Trainium2 cores.

Sharding: core c = 2*b + half handles batch b and heads [8*half, 8*half+8)
(embed slice [512*half, 512*half+512)).  Each core computes its Q/K/V
projections, 8 heads of attention, and a row-parallel out-projection partial
(2048, 1024).  Host unshard: out[b] = partial[2b] + partial[2b+1] + bo.

Per-core device kernel (bf16 matmuls, fp32 accumulation):
  - QT/KT in [d_local, seq] layout (d on partitions) so energy^T = K @ Q^T
    comes out as [k_seq, q_seq] with softmax reductions computable by matmul.
  - softmax without max subtraction (energies are ~N(0,1); exp never overflows)
    with 1/sqrt(64) folded into Wq on the host.
  - exp on the scalar engine straight out of PSUM, bf16 output.
  - V carries an appended ones column so the attn@V matmul (M=65) yields the
    softmax denominator for free in PSUM row 64.
  - normalization: reciprocal_approx_fast of the sums row straight out of
    PSUM, gpsimd partition_broadcast, multiply-on-evict.
  - xq/xk live resident in SBUF (one DMA each); xv is streamed in chunks.
  - scheduling: minimal prologue (QK m=0 + V chunk 0), V chunks 1-3 woven
    into the first attention chunk, QK projections for m+1 woven into m's
    attention, out-projection partially woven into the last m-tile.
"""

import numpy as np
import ml_dtypes

import concourse.bass as bass
import concourse.mybir as mybir
import concourse.tile as tile
import concourse.bacc as bacc
from concourse.bass_utils import run_bass_kernel_spmd

BF16 = mybir.dt.bfloat16
F32 = mybir.dt.float32
NPBF = ml_dtypes.bfloat16

S = 2048          # sequence length
E = 1024          # embed dim
DLOC = 512        # per-core embed slice (8 heads x 64)
HD = 64           # head dim
NHL = 8           # heads per core
KT = E // 128     # 8 contraction tiles for projections
MT = DLOC // 128  # 4 m-tiles of d_local
ST = S // 128     # 16 seq tiles
NCH = S // 512    # 4 seq chunks of 512
EXP = mybir.ActivationFunctionType.Exp


def _build_bass(dump=False):
    nc = bacc.Bacc("TRN2", target_bir_lowering=False, debug=False)

    xqT = nc.dram_tensor("xqT", [E, S], BF16, kind="ExternalInput").ap()
    xkT = nc.dram_tensor("xkT", [E, S], BF16, kind="ExternalInput").ap()
    xvT = nc.dram_tensor("xvT", [E, S], BF16, kind="ExternalInput").ap()
    wq_d = nc.dram_tensor("wq", [E, DLOC], BF16, kind="ExternalInput").ap()
    wk_d = nc.dram_tensor("wk", [E, DLOC], BF16, kind="ExternalInput").ap()
    wv_d = nc.dram_tensor("wv", [E, DLOC], BF16, kind="ExternalInput").ap()
    wo_d = nc.dram_tensor("wo", [DLOC, E], BF16, kind="ExternalInput").ap()
    bq_d = nc.dram_tensor("bq", [128, MT], F32, kind="ExternalInput").ap()
    bk_d = nc.dram_tensor("bk", [128, MT], F32, kind="ExternalInput").ap()
    bv_d = nc.dram_tensor("bv", [1, DLOC], F32, kind="ExternalInput").ap()
    out_d = nc.dram_tensor("out", [S, E], BF16, kind="ExternalOutput").ap()

    xq_r = xqT.rearrange("(kt p) s -> p kt s", p=128)
    xk_r = xkT.rearrange("(kt p) s -> p kt s", p=128)
    xv_r = xvT.rearrange("(kt p) s -> p kt s", p=128)

    with tile.TileContext(nc) as tc:
        _kernel_body(tc, nc, xq_r, xk_r, xv_r, wq_d, wk_d, wv_d, wo_d,
                     bq_d, bk_d, bv_d, out_d, dump=dump)
    nc.compile()
    return nc


def _kernel_body(tc, nc, xq_r, xk_r, xv_r, wq_d, wk_d, wv_d, wo_d,
                 bq_d, bk_d, bv_d, out_d, dump=False):
    from contextlib import ExitStack

    with ExitStack() as ctx:
        wpool = ctx.enter_context(tc.tile_pool(name="weights", bufs=1))
        xpool = ctx.enter_context(tc.tile_pool(name="xstream", bufs=2))
        qkv = ctx.enter_context(tc.tile_pool(name="qkv", bufs=1))
        atp = ctx.enter_context(tc.tile_pool(name="attnt", bufs=4))
        smp = ctx.enter_context(tc.tile_pool(name="small", bufs=2))
        outp = ctx.enter_context(tc.tile_pool(name="outstage", bufs=2))

        # ---- weights / biases / resident inputs to SBUF ----
        wq_sb = wpool.tile([128, KT, DLOC], BF16)
        wk_sb = wpool.tile([128, KT, DLOC], BF16)
        wv_sb = wpool.tile([128, KT, DLOC], BF16)
        wo_sb = wpool.tile([128, MT, E], BF16)
        bq_sb = wpool.tile([128, MT], F32)
        bk_sb = wpool.tile([128, MT], F32)
        bv_row = wpool.tile([1, DLOC], F32)
        bv_bc = wpool.tile([128, DLOC], F32)
        xq_sb = qkv.tile([128, KT, S], BF16)
        xk_sb = qkv.tile([128, KT, S], BF16)

        wq_r = wq_d.rearrange("(kt p) m -> p kt m", p=128)
        wk_r = wk_d.rearrange("(kt p) m -> p kt m", p=128)
        wv_r = wv_d.rearrange("(kt p) m -> p kt m", p=128)

        # DMA issue order == dependency order of the prologue compute:
        # only the m=0 slice of wk/wq gates the prologue projections, so
        # those quarter-weights land first; the rest streams in under the
        # first attention chunk.
        nc.sync.dma_start(bk_sb[:], bk_d)
        # trigger the exp activation-table load during the DMA-bound prologue
        warm = wpool.tile([1, 1], F32)
        nc.scalar.activation(warm[:], bk_sb[0:1, 0:1], EXP)
        nc.sync.dma_start(wk_sb[:, :, 0:128], wk_r[:, :, 0:128])
        for nch in range(NCH):
            nc.sync.dma_start(xk_sb[:, :, bass.ts(nch, 512)],
                              xk_r[:, :, bass.ts(nch, 512)])
        nc.sync.dma_start(wq_sb[:, :, 0:128], wq_r[:, :, 0:128])
        nc.sync.dma_start(bq_sb[:], bq_d)
        for nch in range(2):
            nc.sync.dma_start(xq_sb[:, :, bass.ts(nch, 512)],
                              xq_r[:, :, bass.ts(nch, 512)])
        nc.sync.dma_start(wv_sb[:], wv_r)
        nc.sync.dma_start(bv_row[:], bv_d)
        xv_tiles = {}

        def xv_dma(hc):
            def fn():
                xt = xpool.tile([128, KT, 256], BF16, tag="xs", name="xv_t")
                nc.sync.dma_start(xt[:], xv_r[:, :, bass.ts(hc, 256)])
                xv_tiles[hc] = xt
            return fn

        xv_dma(0)()
        xv_dma(1)()
        nc.sync.dma_start(wk_sb[:, :, 128:DLOC], wk_r[:, :, 128:DLOC])
        nc.sync.dma_start(wq_sb[:, :, 128:DLOC], wq_r[:, :, 128:DLOC])
        for nch in range(2, NCH):
            nc.sync.dma_start(xq_sb[:, :, bass.ts(nch, 512)],
                              xq_r[:, :, bass.ts(nch, 512)])
        nc.sync.dma_start(wo_sb[:], wo_d.rearrange("(mt p) e -> p mt e", p=128))
        nc.gpsimd.partition_broadcast(bv_bc[:], bv_row[:])

        # ---- persistent per-core tensors ----
        QT_sb = qkv.tile([128, MT, S], BF16)        # [d_loc, seq]
        KT_sb = qkv.tile([128, MT, S], BF16)
        V_sb = qkv.tile([128, ST, NHL, HD + 1], BF16)  # ones col at 64
        oT_sb = qkv.tile([128, MT, S], BF16)        # attn out^T (lhsT of outproj)

        nc.vector.memset(V_sb[:, :, :, HD:HD + 1], 1.0)

        # PSUM: 2x [128,1024] energy/proj slots + 2x [65,1024] attn-out slots.
        pe_pool = ctx.enter_context(tc.tile_pool(name="psum_e", bufs=2, space="PSUM"))
        po_pool = ctx.enter_context(tc.tile_pool(name="psum_o", bufs=2, space="PSUM"))

        def v_proj_group(st):
            xv_t = xv_tiles[st // 2]
            ps = pe_pool.tile([128, 1024], F32, tag="pe", name="ps_v")
            for kt in range(KT):
                nc.tensor.matmul(
                    ps[:, 0:512], xv_t[:, kt, bass.ts(st % 2, 128)],
                    wv_sb[:, kt, :], start=(kt == 0), stop=(kt == KT - 1))
            nc.vector.tensor_tensor(
                V_sb[:, st, :, 0:HD],
                ps[:, 0:512].rearrange("p (h d) -> p h d", d=HD),
                bv_bc.rearrange("p (h d) -> p h d", d=HD),
                mybir.AluOpType.add)

        def qk_proj_group(ti, m, nch):
            x_sb = (xq_sb, xk_sb)[ti]
            w_sb = (wq_sb, wk_sb)[ti]
            b_sb = (bq_sb, bk_sb)[ti]
            dst = (QT_sb, KT_sb)[ti]
            ps = pe_pool.tile([128, 1024], F32, tag="pe", name="ps_qk")
            for kt in range(KT):
                nc.tensor.matmul(
                    ps[:, 0:512], w_sb[:, kt, bass.ts(m, 128)],
                    x_sb[:, kt, bass.ts(nch, 512)],
                    start=(kt == 0), stop=(kt == KT - 1))
            nc.vector.tensor_scalar_add(
                dst[:, m, bass.ts(nch, 512)], ps[:, 0:512], b_sb[:, m:m + 1])

        def outproj_group(qt):
            ob = outp.tile([128, E], BF16, tag="ob")
            for ec in range(2):
                ps = pe_pool.tile([128, 1024], F32, tag="pe", name="ps_o")
                for mm in range(MT):
                    nc.tensor.matmul(
                        ps[:, 0:512], oT_sb[:, mm, bass.ts(qt, 128)],
                        wo_sb[:, mm, bass.ts(ec, 512)],
                        start=(mm == 0), stop=(mm == MT - 1))
                nc.vector.tensor_copy(ob[:, bass.ts(ec, 512)], ps[:, 0:512])
            nc.sync.dma_start(out_d[bass.ts(qt, 128), :], ob[:])

        def attn_chunk(m, hs, q0, weave=None, W=1024):
            """One (head, W-q) attention chunk; weave: {kt: [callables]}.

            attn@V trails the energy/exp pipeline by 2 kt steps so the PE
            never waits on the scalar engine's exp latency.
            """
            h = 2 * m + hs
            psl = slice(64 * hs, 64 * hs + 64)
            nqc = W // 512
            po = po_pool.tile([HD + 1, 1024], F32, tag="po")
            pending = []

            def attn_v(pkt, pat):
                for qc in range(nqc):
                    nc.tensor.matmul(
                        po[:, bass.ts(qc, 512)],
                        V_sb[:, pkt, h, :],
                        pat[:, bass.ts(qc, 512)],
                        start=(pkt == 0), stop=(pkt == ST - 1))

            for kt in range(ST):
                pe = pe_pool.tile([128, 1024], F32, tag="pe")
                for qc in range(nqc):
                    nc.tensor.matmul(
                        pe[:, bass.ts(qc, 512)],
                        KT_sb[psl, m, bass.ts(kt, 128)],
                        QT_sb[psl, m, bass.ds(q0 + qc * 512, 512)],
                        start=True, stop=True)
                at = atp.tile([128, 1024], BF16, tag="at")
                nc.scalar.activation(at[:, 0:W], pe[:, 0:W], EXP)
                pending.append((kt, at))
                if len(pending) > 2:
                    attn_v(*pending.pop(0))
                if weave:
                    for fn in weave.get(kt, ()):
                        fn()
            for pkt, pat in pending:
                attn_v(pkt, pat)

            # ---- normalize + evict ----
            # (broadcast the raw sums, then reciprocal+multiply both on the
            # vector engine: the custom-DVE recip output is only ever read by
            # the same engine, dodging its cross-engine completion hazard)
            s_sb = smp.tile([1, 1024], F32, tag="s")
            nc.vector.tensor_copy(s_sb[0:1, 0:W], po[HD:HD + 1, 0:W])
            bc = smp.tile([HD, 1024], F32, tag="bc")
            nc.gpsimd.partition_broadcast(bc[:, 0:W], s_sb[0:1, 0:W])
            nc.vector.reciprocal_approx_fast(bc[:, 0:W], bc[:, 0:W])
            nc.vector.tensor_tensor(
                oT_sb[64 * hs:64 * hs + HD, m, bass.ds(q0, W)],
                po[0:HD, 0:W], bc[:, 0:W], mybir.AluOpType.mult)

        # ---- prologue: K fully, Q first half ----
        for nch in range(NCH):
            qk_proj_group(1, 0, nch)
        for nch in range(2):
            qk_proj_group(0, 0, nch)

        def v_weave(st):
            return lambda: v_proj_group(st)

        def qk_weave(ti, m, nch):
            return lambda: qk_proj_group(ti, m, nch)

        def op_weave(qt):
            return lambda: outproj_group(qt)

        # ---- attention ----
        # chunk 1 (m0,hs0,qh0): all 16 V groups + Q2/Q3 + xv streaming.
        # V seq-tile st is read by attn@V at iteration st+2 (pending depth
        # 2); weaving st at kt=st keeps PE order correct with margin.
        w00 = {}
        for st in range(ST):
            w00.setdefault(min(st, 14), []).append(v_weave(st))
        for hc in range(2, 8):
            w00.setdefault(2 * hc - 4, []).insert(0, xv_dma(hc))
        w00[5] = w00.get(5, []) + [qk_weave(0, 0, 2)]
        w00[10] = w00.get(10, []) + [qk_weave(0, 0, 3)]
        attn_chunk(0, 0, 0, w00)

        # m -> m+1 projection weave, 2-3 groups per chunk, ordered so that
        # K tiles and the first Q half land well before m+1's first chunk.
        def mw(m, spec):
            return {1 + 5 * g: [qk_weave(ti, m + 1, nch)]
                    for g, (ti, nch) in enumerate(spec)}

        attn_chunk(0, 0, 1024, mw(0, [(0, 0), (0, 1), (1, 0)]))
        attn_chunk(0, 1, 0, mw(0, [(1, 1), (1, 2)]))
        attn_chunk(0, 1, 1024, mw(0, [(1, 3), (0, 2), (0, 3)]))
        for m in (1, 2):
            attn_chunk(m, 0, 0, mw(m, [(0, 0), (0, 1)]))
            attn_chunk(m, 0, 1024, mw(m, [(1, 0), (1, 1)]))
            attn_chunk(m, 1, 0, mw(m, [(1, 2), (1, 3)]))
            attn_chunk(m, 1, 1024, mw(m, [(0, 2), (0, 3)]))
        # m=3: qh-outer so the out-projection weaves into the second half;
        # the last head's window is split in two so qt8-11 also weave in.
        attn_chunk(3, 0, 0)
        attn_chunk(3, 1, 0)
        wo0 = {3 + 2 * g: [op_weave(qt)] for g, qt in enumerate(range(6))}
        attn_chunk(3, 0, 1024, wo0)
        attn_chunk(3, 1, 1024, {3: [op_weave(6)], 8: [op_weave(7)]}, W=512)
        attn_chunk(3, 1, 1536,
                   {4 + 3 * g: [op_weave(qt)] for g, qt in enumerate(range(8, 12))},
                   W=512)

        if dump:
            d_qt = nc.dram_tensor("d_qt", [128, MT, S], BF16, kind="ExternalOutput").ap()
            d_kt = nc.dram_tensor("d_kt", [128, MT, S], BF16, kind="ExternalOutput").ap()
            d_v = nc.dram_tensor("d_v", [128, ST, NHL, HD + 1], BF16, kind="ExternalOutput").ap()
            d_ot = nc.dram_tensor("d_ot", [128, MT, S], BF16, kind="ExternalOutput").ap()
            nc.sync.dma_start(d_qt, QT_sb[:])
            nc.sync.dma_start(d_kt, KT_sb[:])
            nc.sync.dma_start(d_v, V_sb[:])
            nc.sync.dma_start(d_ot, oT_sb[:])

        # ---- out-projection tail ----
        for qt in range(12, ST):
            outproj_group(qt)


_CACHED = {}


def _get_bass():
    if "nc" not in _CACHED:
        _CACHED["nc"] = _build_bass()
    return _CACHED["nc"]


def _prep_core_inputs(c, query, key, value, Wq, bq, Wk, bk, Wv, bv, Wo):
    b, half = c // 2, c % 2
    sl = slice(DLOC * half, DLOC * half + DLOC)
    bq_sl = (bq[sl] * 0.125).astype(np.float32).reshape(MT, 128).T.copy()
    bk_sl = bk[sl].astype(np.float32).reshape(MT, 128).T.copy()
    return {
        "xqT": np.ascontiguousarray(query[b].T).astype(NPBF),
        "xkT": np.ascontiguousarray(key[b].T).astype(NPBF),
        "xvT": np.ascontiguousarray(value[b].T).astype(NPBF),
        "wq": np.ascontiguousarray(Wq[sl, :].T * 0.125).astype(NPBF),
        "wk": np.ascontiguousarray(Wk[sl, :].T).astype(NPBF),
        "wv": np.ascontiguousarray(Wv[sl, :].T).astype(NPBF),
        "wo": np.ascontiguousarray(Wo[:, sl].T).astype(NPBF),
        "bq": np.ascontiguousarray(bq_sl),
        "bk": np.ascontiguousarray(bk_sl),
        "bv": bv[sl].astype(np.float32).reshape(1, DLOC).copy(),
    }


def kernel(query, key, value, Wq, bq, Wk, bk, Wv, bv, Wo, bo,
           trace=False, **run_kwargs):
    query = np.asarray(query, np.float32)
    key = np.asarray(key, np.float32)
    value = np.asarray(value, np.float32)
    Wq, Wk, Wv, Wo = (np.asarray(w, np.float32) for w in (Wq, Wk, Wv, Wo))
    bq, bk, bv, bo = (np.asarray(x, np.float32) for x in (bq, bk, bv, bo))

    nc = _get_bass()
    in_maps = [_prep_core_inputs(c, query, key, value, Wq, bq, Wk, bk, Wv, bv, Wo)
               for c in range(8)]
    res = run_bass_kernel_spmd(nc, in_maps, core_ids=list(range(8)),
                               trace=trace, **run_kwargs)
    _CACHED["last_result"] = res

    B = query.shape[0]
    out = np.empty((B, S, E), np.float32)
    for b in range(B):
        out[b] = (res.results[2 * b]["out"].astype(np.float32)
                  + res.results[2 * b + 1]["out"].astype(np.float32) + bo)
    return out


# revision 25
# speedup vs baseline: 1.0270x; 1.0213x over previous
"""Multi-head attention (B=4, S=2048, E=1024, 16 heads x 64) on 8 Trainium2 cores.

Sharding: core c = 2*b + half handles batch b and heads [8*half, 8*half+8)
(embed slice [512*half, 512*half+512)).  Each core computes its Q/K/V
projections, 8 heads of attention, and a row-parallel out-projection partial
(2048, 1024).  Host unshard: out[b] = partial[2b] + partial[2b+1] + bo.

Per-core device kernel (bf16 matmuls, fp32 accumulation):
  - QT/KT in [d_local, seq] layout (d on partitions) so energy^T = K @ Q^T
    comes out as [k_seq, q_seq] with softmax reductions computable by matmul.
  - softmax without max subtraction (energies are ~N(0,1); exp never overflows)
    with 1/sqrt(64) folded into Wq on the host.
  - exp on the scalar engine straight out of PSUM, bf16 output.
  - V carries an appended ones column so the attn@V matmul (M=65) yields the
    softmax denominator for free in PSUM row 64.
  - normalization: reciprocal_approx_fast of the sums row straight out of
    PSUM, gpsimd partition_broadcast, multiply-on-evict.
  - xq/xk live resident in SBUF (one DMA each); xv is streamed in chunks.
  - scheduling: minimal prologue (QK m=0 + V chunk 0), V chunks 1-3 woven
    into the first attention chunk, QK projections for m+1 woven into m's
    attention, out-projection partially woven into the last m-tile.
"""

import numpy as np
import ml_dtypes

import concourse.bass as bass
import concourse.mybir as mybir
import concourse.tile as tile
import concourse.bacc as bacc
from concourse.bass_utils import run_bass_kernel_spmd

BF16 = mybir.dt.bfloat16
F32 = mybir.dt.float32
NPBF = ml_dtypes.bfloat16

S = 2048          # sequence length
E = 1024          # embed dim
DLOC = 512        # per-core embed slice (8 heads x 64)
HD = 64           # head dim
NHL = 8           # heads per core
KT = E // 128     # 8 contraction tiles for projections
MT = DLOC // 128  # 4 m-tiles of d_local
ST = S // 128     # 16 seq tiles
NCH = S // 512    # 4 seq chunks of 512
EXP = mybir.ActivationFunctionType.Exp


def _build_bass(dump=False):
    nc = bacc.Bacc("TRN2", target_bir_lowering=False, debug=False)

    xqT = nc.dram_tensor("xqT", [E, S], BF16, kind="ExternalInput").ap()
    xkT = nc.dram_tensor("xkT", [E, S], BF16, kind="ExternalInput").ap()
    xvT = nc.dram_tensor("xvT", [E, S], BF16, kind="ExternalInput").ap()
    wq_d = nc.dram_tensor("wq", [E, DLOC], BF16, kind="ExternalInput").ap()
    wk_d = nc.dram_tensor("wk", [E, DLOC], BF16, kind="ExternalInput").ap()
    wv_d = nc.dram_tensor("wv", [E, DLOC], BF16, kind="ExternalInput").ap()
    wo_d = nc.dram_tensor("wo", [DLOC, E], BF16, kind="ExternalInput").ap()
    bq_d = nc.dram_tensor("bq", [128, MT], F32, kind="ExternalInput").ap()
    bk_d = nc.dram_tensor("bk", [128, MT], F32, kind="ExternalInput").ap()
    bv_d = nc.dram_tensor("bv", [1, DLOC], F32, kind="ExternalInput").ap()
    out_d = nc.dram_tensor("out", [S, E], BF16, kind="ExternalOutput").ap()

    xq_r = xqT.rearrange("(kt p) s -> p kt s", p=128)
    xk_r = xkT.rearrange("(kt p) s -> p kt s", p=128)
    xv_r = xvT.rearrange("(kt p) s -> p kt s", p=128)

    with tile.TileContext(nc) as tc:
        _kernel_body(tc, nc, xq_r, xk_r, xv_r, wq_d, wk_d, wv_d, wo_d,
                     bq_d, bk_d, bv_d, out_d, dump=dump)
    nc.compile()
    return nc


def _kernel_body(tc, nc, xq_r, xk_r, xv_r, wq_d, wk_d, wv_d, wo_d,
                 bq_d, bk_d, bv_d, out_d, dump=False):
    from contextlib import ExitStack

    with ExitStack() as ctx:
        wpool = ctx.enter_context(tc.tile_pool(name="weights", bufs=1))
        xpool = ctx.enter_context(tc.tile_pool(name="xstream", bufs=2))
        qkv = ctx.enter_context(tc.tile_pool(name="qkv", bufs=1))
        atp = ctx.enter_context(tc.tile_pool(name="attnt", bufs=4))
        smp = ctx.enter_context(tc.tile_pool(name="small", bufs=2))
        outp = ctx.enter_context(tc.tile_pool(name="outstage", bufs=2))

        # ---- weights / biases / resident inputs to SBUF ----
        wq_sb = wpool.tile([128, KT, DLOC], BF16)
        wk_sb = wpool.tile([128, KT, DLOC], BF16)
        wv_sb = wpool.tile([128, KT, DLOC], BF16)
        wo_sb = wpool.tile([128, MT, E], BF16)
        bq_sb = wpool.tile([128, MT], F32)
        bk_sb = wpool.tile([128, MT], F32)
        bv_row = wpool.tile([1, DLOC], F32)
        bv_bc = wpool.tile([128, DLOC], F32)
        xq_sb = qkv.tile([128, KT, S], BF16)
        xk_sb = qkv.tile([128, KT, S], BF16)

        wq_r = wq_d.rearrange("(kt p) m -> p kt m", p=128)
        wk_r = wk_d.rearrange("(kt p) m -> p kt m", p=128)
        wv_r = wv_d.rearrange("(kt p) m -> p kt m", p=128)

        # DMA issue order == dependency order of the prologue compute:
        # only the m=0 slice of wk/wq gates the prologue projections, so
        # those quarter-weights land first; the rest streams in under the
        # first attention chunk.
        nc.sync.dma_start(bk_sb[:], bk_d)
        # trigger the exp activation-table load during the DMA-bound prologue
        warm = wpool.tile([1, 1], F32)
        nc.scalar.activation(warm[:], bk_sb[0:1, 0:1], EXP)
        nc.sync.dma_start(wk_sb[:, :, 0:128], wk_r[:, :, 0:128])
        for nch in range(NCH):
            nc.sync.dma_start(xk_sb[:, :, bass.ts(nch, 512)],
                              xk_r[:, :, bass.ts(nch, 512)])
        nc.sync.dma_start(wq_sb[:, :, 0:128], wq_r[:, :, 0:128])
        nc.sync.dma_start(bq_sb[:], bq_d)
        for nch in range(2):
            nc.sync.dma_start(xq_sb[:, :, bass.ts(nch, 512)],
                              xq_r[:, :, bass.ts(nch, 512)])
        nc.sync.dma_start(wv_sb[:], wv_r)
        nc.sync.dma_start(bv_row[:], bv_d)
        xv_tiles = {}

        def xv_dma(hc):
            def fn():
                xt = xpool.tile([128, KT, 256], BF16, tag="xs", name="xv_t")
                nc.sync.dma_start(xt[:], xv_r[:, :, bass.ts(hc, 256)])
                xv_tiles[hc] = xt
            return fn

        xv_dma(0)()
        xv_dma(1)()
        xv_dma(2)()
        for nch in range(2, NCH):
            nc.sync.dma_start(xq_sb[:, :, bass.ts(nch, 512)],
                              xq_r[:, :, bass.ts(nch, 512)])
        # h3 reuses h0's buffer: its DMA self-gates on chunk 1's first V
        # groups, and everything below is only needed much later.
        xv_dma(3)()
        nc.sync.dma_start(wk_sb[:, :, 128:DLOC], wk_r[:, :, 128:DLOC])
        nc.sync.dma_start(wq_sb[:, :, 128:DLOC], wq_r[:, :, 128:DLOC])
        nc.sync.dma_start(wo_sb[:], wo_d.rearrange("(mt p) e -> p mt e", p=128))
        nc.gpsimd.partition_broadcast(bv_bc[:], bv_row[:])

        # ---- persistent per-core tensors ----
        QT_sb = qkv.tile([128, MT, S], BF16)        # [d_loc, seq]
        KT_sb = qkv.tile([128, MT, S], BF16)
        V_sb = qkv.tile([128, ST, NHL, HD + 1], BF16)  # ones col at 64
        oT_sb = qkv.tile([128, MT, S], BF16)        # attn out^T (lhsT of outproj)

        nc.vector.memset(V_sb[:, :, :, HD:HD + 1], 1.0)

        # PSUM: 2x [128,1024] energy/proj slots + 2x [65,1024] attn-out slots.
        pe_pool = ctx.enter_context(tc.tile_pool(name="psum_e", bufs=2, space="PSUM"))
        po_pool = ctx.enter_context(tc.tile_pool(name="psum_o", bufs=2, space="PSUM"))

        def v_proj_group(st):
            xv_t = xv_tiles[st // 2]
            ps = pe_pool.tile([128, 1024], F32, tag="pe", name="ps_v")
            for kt in range(KT):
                nc.tensor.matmul(
                    ps[:, 0:512], xv_t[:, kt, bass.ts(st % 2, 128)],
                    wv_sb[:, kt, :], start=(kt == 0), stop=(kt == KT - 1))
            nc.vector.tensor_tensor(
                V_sb[:, st, :, 0:HD],
                ps[:, 0:512].rearrange("p (h d) -> p h d", d=HD),
                bv_bc.rearrange("p (h d) -> p h d", d=HD),
                mybir.AluOpType.add)

        def qk_proj_group(ti, m, nch):
            x_sb = (xq_sb, xk_sb)[ti]
            w_sb = (wq_sb, wk_sb)[ti]
            b_sb = (bq_sb, bk_sb)[ti]
            dst = (QT_sb, KT_sb)[ti]
            ps = pe_pool.tile([128, 1024], F32, tag="pe", name="ps_qk")
            for kt in range(KT):
                nc.tensor.matmul(
                    ps[:, 0:512], w_sb[:, kt, bass.ts(m, 128)],
                    x_sb[:, kt, bass.ts(nch, 512)],
                    start=(kt == 0), stop=(kt == KT - 1))
            nc.vector.tensor_scalar_add(
                dst[:, m, bass.ts(nch, 512)], ps[:, 0:512], b_sb[:, m:m + 1])

        def outproj_group(qt):
            ob = outp.tile([128, E], BF16, tag="ob")
            for ec in range(2):
                ps = pe_pool.tile([128, 1024], F32, tag="pe", name="ps_o")
                for mm in range(MT):
                    nc.tensor.matmul(
                        ps[:, 0:512], oT_sb[:, mm, bass.ts(qt, 128)],
                        wo_sb[:, mm, bass.ts(ec, 512)],
                        start=(mm == 0), stop=(mm == MT - 1))
                nc.vector.tensor_copy(ob[:, bass.ts(ec, 512)], ps[:, 0:512])
            nc.sync.dma_start(out_d[bass.ts(qt, 128), :], ob[:])

        def attn_chunk(m, hs, q0, weave=None, W=1024):
            """One (head, W-q) attention chunk; weave: {kt: [callables]}.

            attn@V trails the energy/exp pipeline by 2 kt steps so the PE
            never waits on the scalar engine's exp latency.
            """
            h = 2 * m + hs
            psl = slice(64 * hs, 64 * hs + 64)
            nqc = W // 512
            po = po_pool.tile([HD + 1, 1024], F32, tag="po")
            pending = []

            def attn_v(pkt, pat):
                for qc in range(nqc):
                    nc.tensor.matmul(
                        po[:, bass.ts(qc, 512)],
                        V_sb[:, pkt, h, :],
                        pat[:, bass.ts(qc, 512)],
                        start=(pkt == 0), stop=(pkt == ST - 1))

            for kt in range(ST):
                pe = pe_pool.tile([128, 1024], F32, tag="pe")
                for qc in range(nqc):
                    nc.tensor.matmul(
                        pe[:, bass.ts(qc, 512)],
                        KT_sb[psl, m, bass.ts(kt, 128)],
                        QT_sb[psl, m, bass.ds(q0 + qc * 512, 512)],
                        start=True, stop=True)
                at = atp.tile([128, 1024], BF16, tag="at")
                nc.scalar.activation(at[:, 0:W], pe[:, 0:W], EXP)
                pending.append((kt, at))
                if len(pending) > 2:
                    attn_v(*pending.pop(0))
                if weave:
                    for fn in weave.get(kt, ()):
                        fn()
            for pkt, pat in pending:
                attn_v(pkt, pat)

            # ---- normalize + evict ----
            # (broadcast the raw sums, then reciprocal+multiply both on the
            # vector engine: the custom-DVE recip output is only ever read by
            # the same engine, dodging its cross-engine completion hazard)
            s_sb = smp.tile([1, 1024], F32, tag="s")
            nc.vector.tensor_copy(s_sb[0:1, 0:W], po[HD:HD + 1, 0:W])
            bc = smp.tile([HD, 1024], F32, tag="bc")
            nc.gpsimd.partition_broadcast(bc[:, 0:W], s_sb[0:1, 0:W])
            nc.vector.reciprocal_approx_fast(bc[:, 0:W], bc[:, 0:W])
            nc.vector.tensor_tensor(
                oT_sb[64 * hs:64 * hs + HD, m, bass.ds(q0, W)],
                po[0:HD, 0:W], bc[:, 0:W], mybir.AluOpType.mult)

        # ---- prologue: K fully, Q first half ----
        for nch in range(NCH):
            qk_proj_group(1, 0, nch)
        for nch in range(2):
            qk_proj_group(0, 0, nch)

        def v_weave(st):
            return lambda: v_proj_group(st)

        def qk_weave(ti, m, nch):
            return lambda: qk_proj_group(ti, m, nch)

        def op_weave(qt):
            return lambda: outproj_group(qt)

        # ---- attention ----
        # chunk 1 (m0,hs0,qh0): all 16 V groups + Q2/Q3 + xv streaming.
        # V seq-tile st is read by attn@V at iteration st+2 (pending depth
        # 2); weaving st at kt=st keeps PE order correct with margin.
        w00 = {}
        for st in range(ST):
            w00.setdefault(min(st, 14), []).append(v_weave(st))
        for hc in range(4, 8):
            w00.setdefault(2 * hc - 4, []).insert(0, xv_dma(hc))
        w00[5] = w00.get(5, []) + [qk_weave(0, 0, 2)]
        w00[10] = w00.get(10, []) + [qk_weave(0, 0, 3)]
        attn_chunk(0, 0, 0, w00)

        # m -> m+1 projection weave, 2-3 groups per chunk, ordered so that
        # K tiles and the first Q half land well before m+1's first chunk.
        def mw(m, spec):
            return {1 + 5 * g: [qk_weave(ti, m + 1, nch)]
                    for g, (ti, nch) in enumerate(spec)}

        attn_chunk(0, 0, 1024, mw(0, [(0, 0), (0, 1), (1, 0)]))
        attn_chunk(0, 1, 0, mw(0, [(1, 1), (1, 2)]))
        attn_chunk(0, 1, 1024, mw(0, [(1, 3), (0, 2), (0, 3)]))
        for m in (1, 2):
            attn_chunk(m, 0, 0, mw(m, [(0, 0), (0, 1)]))
            attn_chunk(m, 0, 1024, mw(m, [(1, 0), (1, 1)]))
            attn_chunk(m, 1, 0, mw(m, [(1, 2), (1, 3)]))
            attn_chunk(m, 1, 1024, mw(m, [(0, 2), (0, 3)]))
        # m=3: qh-outer so the out-projection weaves into the second half.
        attn_chunk(3, 0, 0)
        attn_chunk(3, 1, 0)
        wo0 = {3 + 2 * g: [op_weave(qt)] for g, qt in enumerate(range(6))}
        wo1 = {3 + 4 * g: [op_weave(qt)] for g, qt in enumerate(range(6, 8))}
        attn_chunk(3, 0, 1024, wo0)
        attn_chunk(3, 1, 1024, wo1)

        if dump:
            d_qt = nc.dram_tensor("d_qt", [128, MT, S], BF16, kind="ExternalOutput").ap()
            d_kt = nc.dram_tensor("d_kt", [128, MT, S], BF16, kind="ExternalOutput").ap()
            d_v = nc.dram_tensor("d_v", [128, ST, NHL, HD + 1], BF16, kind="ExternalOutput").ap()
            d_ot = nc.dram_tensor("d_ot", [128, MT, S], BF16, kind="ExternalOutput").ap()
            nc.sync.dma_start(d_qt, QT_sb[:])
            nc.sync.dma_start(d_kt, KT_sb[:])
            nc.sync.dma_start(d_v, V_sb[:])
            nc.sync.dma_start(d_ot, oT_sb[:])

        # ---- out-projection tail ----
        for qt in range(8, ST):
            outproj_group(qt)


_CACHED = {}


def _get_bass():
    if "nc" not in _CACHED:
        _CACHED["nc"] = _build_bass()
    return _CACHED["nc"]


def _prep_core_inputs(c, query, key, value, Wq, bq, Wk, bk, Wv, bv, Wo):
    b, half = c // 2, c % 2
    sl = slice(DLOC * half, DLOC * half + DLOC)
    bq_sl = (bq[sl] * 0.125).astype(np.float32).reshape(MT, 128).T.copy()
    bk_sl = bk[sl].astype(np.float32).reshape(MT, 128).T.copy()
    return {
        "xqT": np.ascontiguousarray(query[b].T).astype(NPBF),
        "xkT": np.ascontiguousarray(key[b].T).astype(NPBF),
        "xvT": np.ascontiguousarray(value[b].T).astype(NPBF),
        "wq": np.ascontiguousarray(Wq[sl, :].T * 0.125).astype(NPBF),
        "wk": np.ascontiguousarray(Wk[sl, :].T).astype(NPBF),
        "wv": np.ascontiguousarray(Wv[sl, :].T).astype(NPBF),
        "wo": np.ascontiguousarray(Wo[:, sl].T).astype(NPBF),
        "bq": np.ascontiguousarray(bq_sl),
        "bk": np.ascontiguousarray(bk_sl),
        "bv": bv[sl].astype(np.float32).reshape(1, DLOC).copy(),
    }


def kernel(query, key, value, Wq, bq, Wk, bk, Wv, bv, Wo, bo,
           trace=False, **run_kwargs):
    query = np.asarray(query, np.float32)
    key = np.asarray(key, np.float32)
    value = np.asarray(value, np.float32)
    Wq, Wk, Wv, Wo = (np.asarray(w, np.float32) for w in (Wq, Wk, Wv, Wo))
    bq, bk, bv, bo = (np.asarray(x, np.float32) for x in (bq, bk, bv, bo))

    nc = _get_bass()
    in_maps = [_prep_core_inputs(c, query, key, value, Wq, bq, Wk, bk, Wv, bv, Wo)
               for c in range(8)]
    res = run_bass_kernel_spmd(nc, in_maps, core_ids=list(range(8)),
                               trace=trace, **run_kwargs)
    _CACHED["last_result"] = res

    B = query.shape[0]
    out = np.empty((B, S, E), np.float32)
    for b in range(B):
        out[b] = (res.results[2 * b]["out"].astype(np.float32)
                  + res.results[2 * b + 1]["out"].astype(np.float32) + bo)
    return out


# revision 29
# speedup vs baseline: 1.0278x; 1.0008x over previous
"""Multi-head attention (B=4, S=2048, E=1024, 16 heads x 64) on 8 Trainium2 cores.

Sharding: core c = 2*b + half handles batch b and heads [8*half, 8*half+8)
(embed slice [512*half, 512*half+512)).  Each core computes its Q/K/V
projections, 8 heads of attention, and a row-parallel out-projection partial
(2048, 1024).  Host unshard: out[b] = partial[2b] + partial[2b+1] + bo.

Per-core device kernel (bf16 matmuls, fp32 accumulation):
  - QT/KT in [d_local, seq] layout (d on partitions) so energy^T = K @ Q^T
    comes out as [k_seq, q_seq] with softmax reductions computable by matmul.
  - softmax without max subtraction (energies are ~N(0,1); exp never overflows)
    with 1/sqrt(64) folded into Wq on the host.
  - exp on the scalar engine straight out of PSUM, bf16 output.
  - V carries an appended ones column so the attn@V matmul (M=65) yields the
    softmax denominator for free in PSUM row 64.
  - normalization: reciprocal_approx_fast of the sums row straight out of
    PSUM, gpsimd partition_broadcast, multiply-on-evict.
  - xq/xk live resident in SBUF (one DMA each); xv is streamed in chunks.
  - scheduling: minimal prologue (QK m=0 + V chunk 0), V chunks 1-3 woven
    into the first attention chunk, QK projections for m+1 woven into m's
    attention, out-projection partially woven into the last m-tile.
"""

import numpy as np
import ml_dtypes

import concourse.bass as bass
import concourse.mybir as mybir
import concourse.tile as tile
import concourse.bacc as bacc
from concourse.bass_utils import run_bass_kernel_spmd

BF16 = mybir.dt.bfloat16
F32 = mybir.dt.float32
NPBF = ml_dtypes.bfloat16

S = 2048          # sequence length
E = 1024          # embed dim
DLOC = 512        # per-core embed slice (8 heads x 64)
HD = 64           # head dim
NHL = 8           # heads per core
KT = E // 128     # 8 contraction tiles for projections
MT = DLOC // 128  # 4 m-tiles of d_local
ST = S // 128     # 16 seq tiles
NCH = S // 512    # 4 seq chunks of 512
EXP = mybir.ActivationFunctionType.Exp


def _build_bass(dump=False):
    nc = bacc.Bacc("TRN2", target_bir_lowering=False, debug=False)

    xqT = nc.dram_tensor("xqT", [E, S], BF16, kind="ExternalInput").ap()
    xkT = nc.dram_tensor("xkT", [E, S], BF16, kind="ExternalInput").ap()
    xvT = nc.dram_tensor("xvT", [E, S], BF16, kind="ExternalInput").ap()
    wq_d = nc.dram_tensor("wq", [E, DLOC], BF16, kind="ExternalInput").ap()
    wk_d = nc.dram_tensor("wk", [E, DLOC], BF16, kind="ExternalInput").ap()
    wv_d = nc.dram_tensor("wv", [E, DLOC], BF16, kind="ExternalInput").ap()
    wo_d = nc.dram_tensor("wo", [DLOC, E], BF16, kind="ExternalInput").ap()
    bq_d = nc.dram_tensor("bq", [128, MT], F32, kind="ExternalInput").ap()
    bk_d = nc.dram_tensor("bk", [128, MT], F32, kind="ExternalInput").ap()
    bv_d = nc.dram_tensor("bv", [1, DLOC], F32, kind="ExternalInput").ap()
    out_d = nc.dram_tensor("out", [S, E], BF16, kind="ExternalOutput").ap()

    xq_r = xqT.rearrange("(kt p) s -> p kt s", p=128)
    xk_r = xkT.rearrange("(kt p) s -> p kt s", p=128)
    xv_r = xvT.rearrange("(kt p) s -> p kt s", p=128)

    with tile.TileContext(nc) as tc:
        _kernel_body(tc, nc, xq_r, xk_r, xv_r, wq_d, wk_d, wv_d, wo_d,
                     bq_d, bk_d, bv_d, out_d, dump=dump)
    nc.compile()
    return nc


def _kernel_body(tc, nc, xq_r, xk_r, xv_r, wq_d, wk_d, wv_d, wo_d,
                 bq_d, bk_d, bv_d, out_d, dump=False):
    from contextlib import ExitStack

    with ExitStack() as ctx:
        wpool = ctx.enter_context(tc.tile_pool(name="weights", bufs=1))
        xpool = ctx.enter_context(tc.tile_pool(name="xstream", bufs=2))
        qkv = ctx.enter_context(tc.tile_pool(name="qkv", bufs=1))
        atp = ctx.enter_context(tc.tile_pool(name="attnt", bufs=4))
        smp = ctx.enter_context(tc.tile_pool(name="small", bufs=2))
        outp = ctx.enter_context(tc.tile_pool(name="outstage", bufs=2))

        # ---- weights / biases / resident inputs to SBUF ----
        wq_sb = wpool.tile([128, KT, DLOC], BF16)
        wk_sb = wpool.tile([128, KT, DLOC], BF16)
        wv_sb = wpool.tile([128, KT, DLOC], BF16)
        wo_sb = wpool.tile([128, MT, E], BF16)
        bq_sb = wpool.tile([128, MT], F32)
        bk_sb = wpool.tile([128, MT], F32)
        bv_row = wpool.tile([1, DLOC], F32)
        bv_bc = wpool.tile([128, DLOC], F32)
        xq_sb = qkv.tile([128, KT, S], BF16)
        xk_sb = qkv.tile([128, KT, S], BF16)

        wq_r = wq_d.rearrange("(kt p) m -> p kt m", p=128)
        wk_r = wk_d.rearrange("(kt p) m -> p kt m", p=128)
        wv_r = wv_d.rearrange("(kt p) m -> p kt m", p=128)

        # DMA issue order == dependency order of the prologue compute:
        # only the m=0 slice of wk/wq gates the prologue projections, so
        # those quarter-weights land first; the rest streams in under the
        # first attention chunk.
        nc.sync.dma_start(bk_sb[:], bk_d)
        # trigger the exp activation-table load during the DMA-bound prologue
        warm = wpool.tile([1, 1], F32)
        nc.scalar.activation(warm[:], bk_sb[0:1, 0:1], EXP)
        nc.sync.dma_start(wk_sb[:, :, 0:128], wk_r[:, :, 0:128])
        for nch in range(2):
            nc.sync.dma_start(xk_sb[:, :, bass.ts(nch, 512)],
                              xk_r[:, :, bass.ts(nch, 512)])
        nc.sync.dma_start(wq_sb[:, :, 0:128], wq_r[:, :, 0:128])
        nc.sync.dma_start(bq_sb[:], bq_d)
        for nch in range(2):
            nc.sync.dma_start(xq_sb[:, :, bass.ts(nch, 512)],
                              xq_r[:, :, bass.ts(nch, 512)])
        nc.sync.dma_start(wv_sb[:], wv_r)
        nc.sync.dma_start(bv_row[:], bv_d)
        xv_tiles = {}

        def xv_dma(hc):
            def fn():
                xt = xpool.tile([128, KT, 256], BF16, tag="xs", name="xv_t")
                nc.sync.dma_start(xt[:], xv_r[:, :, bass.ts(hc, 256)])
                xv_tiles[hc] = xt
            return fn

        xv_dma(0)()
        xv_dma(1)()
        xv_dma(2)()
        for nch in range(2, NCH):
            nc.sync.dma_start(xk_sb[:, :, bass.ts(nch, 512)],
                              xk_r[:, :, bass.ts(nch, 512)])
        for nch in range(2, NCH):
            nc.sync.dma_start(xq_sb[:, :, bass.ts(nch, 512)],
                              xq_r[:, :, bass.ts(nch, 512)])
        # h3 reuses h0's buffer: its DMA self-gates on chunk 1's first V
        # groups, and everything below is only needed much later.
        xv_dma(3)()
        nc.sync.dma_start(wk_sb[:, :, 128:DLOC], wk_r[:, :, 128:DLOC])
        nc.sync.dma_start(wq_sb[:, :, 128:DLOC], wq_r[:, :, 128:DLOC])
        nc.sync.dma_start(wo_sb[:], wo_d.rearrange("(mt p) e -> p mt e", p=128))
        nc.gpsimd.partition_broadcast(bv_bc[:], bv_row[:])

        # ---- persistent per-core tensors ----
        QT_sb = qkv.tile([128, MT, S], BF16)        # [d_loc, seq]
        KT_sb = qkv.tile([128, MT, S], BF16)
        V_sb = qkv.tile([128, ST, NHL, HD + 1], BF16)  # ones col at 64
        oT_sb = qkv.tile([128, MT, S], BF16)        # attn out^T (lhsT of outproj)

        nc.vector.memset(V_sb[:, :, :, HD:HD + 1], 1.0)

        # PSUM: 2x [128,1024] energy/proj slots + 2x [65,1024] attn-out slots.
        pe_pool = ctx.enter_context(tc.tile_pool(name="psum_e", bufs=2, space="PSUM"))
        po_pool = ctx.enter_context(tc.tile_pool(name="psum_o", bufs=2, space="PSUM"))

        def v_proj_group(st):
            xv_t = xv_tiles[st // 2]
            ps = pe_pool.tile([128, 1024], F32, tag="pe", name="ps_v")
            for kt in range(KT):
                nc.tensor.matmul(
                    ps[:, 0:512], xv_t[:, kt, bass.ts(st % 2, 128)],
                    wv_sb[:, kt, :], start=(kt == 0), stop=(kt == KT - 1))
            nc.vector.tensor_tensor(
                V_sb[:, st, :, 0:HD],
                ps[:, 0:512].rearrange("p (h d) -> p h d", d=HD),
                bv_bc.rearrange("p (h d) -> p h d", d=HD),
                mybir.AluOpType.add)

        def qk_proj_group(ti, m, nch):
            x_sb = (xq_sb, xk_sb)[ti]
            w_sb = (wq_sb, wk_sb)[ti]
            b_sb = (bq_sb, bk_sb)[ti]
            dst = (QT_sb, KT_sb)[ti]
            ps = pe_pool.tile([128, 1024], F32, tag="pe", name="ps_qk")
            for kt in range(KT):
                nc.tensor.matmul(
                    ps[:, 0:512], w_sb[:, kt, bass.ts(m, 128)],
                    x_sb[:, kt, bass.ts(nch, 512)],
                    start=(kt == 0), stop=(kt == KT - 1))
            nc.vector.tensor_scalar_add(
                dst[:, m, bass.ts(nch, 512)], ps[:, 0:512], b_sb[:, m:m + 1])

        def outproj_group(qt):
            ob = outp.tile([128, E], BF16, tag="ob")
            for ec in range(2):
                ps = pe_pool.tile([128, 1024], F32, tag="pe", name="ps_o")
                for mm in range(MT):
                    nc.tensor.matmul(
                        ps[:, 0:512], oT_sb[:, mm, bass.ts(qt, 128)],
                        wo_sb[:, mm, bass.ts(ec, 512)],
                        start=(mm == 0), stop=(mm == MT - 1))
                nc.vector.tensor_copy(ob[:, bass.ts(ec, 512)], ps[:, 0:512])
            nc.sync.dma_start(out_d[bass.ts(qt, 128), :], ob[:])

        def attn_chunk(m, hs, q0, weave=None, W=1024):
            """One (head, W-q) attention chunk; weave: {kt: [callables]}.

            attn@V trails the energy/exp pipeline by 2 kt steps so the PE
            never waits on the scalar engine's exp latency.
            """
            h = 2 * m + hs
            psl = slice(64 * hs, 64 * hs + 64)
            nqc = W // 512
            po = po_pool.tile([HD + 1, 1024], F32, tag="po")
            pending = []

            def attn_v(pkt, pat):
                for qc in range(nqc):
                    nc.tensor.matmul(
                        po[:, bass.ts(qc, 512)],
                        V_sb[:, pkt, h, :],
                        pat[:, bass.ts(qc, 512)],
                        start=(pkt == 0), stop=(pkt == ST - 1))

            for kt in range(ST):
                pe = pe_pool.tile([128, 1024], F32, tag="pe")
                for qc in range(nqc):
                    nc.tensor.matmul(
                        pe[:, bass.ts(qc, 512)],
                        KT_sb[psl, m, bass.ts(kt, 128)],
                        QT_sb[psl, m, bass.ds(q0 + qc * 512, 512)],
                        start=True, stop=True)
                at = atp.tile([128, 1024], BF16, tag="at")
                nc.scalar.activation(at[:, 0:W], pe[:, 0:W], EXP)
                pending.append((kt, at))
                if len(pending) > 2:
                    attn_v(*pending.pop(0))
                if weave:
                    for fn in weave.get(kt, ()):
                        fn()
            for pkt, pat in pending:
                attn_v(pkt, pat)

            # ---- normalize + evict ----
            # (broadcast the raw sums, then reciprocal+multiply both on the
            # vector engine: the custom-DVE recip output is only ever read by
            # the same engine, dodging its cross-engine completion hazard)
            s_sb = smp.tile([1, 1024], F32, tag="s")
            nc.vector.tensor_copy(s_sb[0:1, 0:W], po[HD:HD + 1, 0:W])
            bc = smp.tile([HD, 1024], F32, tag="bc")
            nc.gpsimd.partition_broadcast(bc[:, 0:W], s_sb[0:1, 0:W])
            nc.vector.reciprocal_approx_fast(bc[:, 0:W], bc[:, 0:W])
            nc.vector.tensor_tensor(
                oT_sb[64 * hs:64 * hs + HD, m, bass.ds(q0, W)],
                po[0:HD, 0:W], bc[:, 0:W], mybir.AluOpType.mult)

        # ---- prologue: K and Q first halves; K2/K3 weave into chunk 1 ----
        for nch in range(2):
            qk_proj_group(1, 0, nch)
        for nch in range(2):
            qk_proj_group(0, 0, nch)

        def v_weave(st):
            return lambda: v_proj_group(st)

        def qk_weave(ti, m, nch):
            return lambda: qk_proj_group(ti, m, nch)

        def op_weave(qt):
            return lambda: outproj_group(qt)

        # ---- attention ----
        # chunk 1 (m0,hs0,qh0): all 16 V groups + Q2/Q3 + xv streaming.
        # V seq-tile st is read by attn@V at iteration st+2 (pending depth
        # 2); weaving st at kt=st keeps PE order correct with margin.
        w00 = {}
        for st in range(ST):
            w00.setdefault(min(st, 14), []).append(v_weave(st))
        for hc in range(4, 8):
            w00.setdefault(2 * hc - 4, []).insert(0, xv_dma(hc))
        w00[0] = w00.get(0, []) + [qk_weave(1, 0, 2)]   # K2: read from kt=8
        w00[2] = w00.get(2, []) + [qk_weave(1, 0, 3)]   # K3: read from kt=12
        w00[5] = w00.get(5, []) + [qk_weave(0, 0, 2)]
        w00[10] = w00.get(10, []) + [qk_weave(0, 0, 3)]
        attn_chunk(0, 0, 0, w00)

        # m -> m+1 projection weave, 2-3 groups per chunk, ordered so that
        # K tiles and the first Q half land well before m+1's first chunk.
        def mw(m, spec):
            return {1 + 5 * g: [qk_weave(ti, m + 1, nch)]
                    for g, (ti, nch) in enumerate(spec)}

        attn_chunk(0, 0, 1024, mw(0, [(0, 0), (0, 1), (1, 0)]))
        attn_chunk(0, 1, 0, mw(0, [(1, 1), (1, 2)]))
        attn_chunk(0, 1, 1024, mw(0, [(1, 3), (0, 2), (0, 3)]))
        for m in (1, 2):
            attn_chunk(m, 0, 0, mw(m, [(0, 0), (0, 1)]))
            attn_chunk(m, 0, 1024, mw(m, [(1, 0), (1, 1)]))
            attn_chunk(m, 1, 0, mw(m, [(1, 2), (1, 3)]))
            attn_chunk(m, 1, 1024, mw(m, [(0, 2), (0, 3)]))
        # m=3: qh-outer so the out-projection weaves into the second half.
        attn_chunk(3, 0, 0)
        attn_chunk(3, 1, 0)
        wo0 = {3 + 2 * g: [op_weave(qt)] for g, qt in enumerate(range(6))}
        wo1 = {3 + 4 * g: [op_weave(qt)] for g, qt in enumerate(range(6, 8))}
        attn_chunk(3, 0, 1024, wo0)
        attn_chunk(3, 1, 1024, wo1)

        if dump:
            d_qt = nc.dram_tensor("d_qt", [128, MT, S], BF16, kind="ExternalOutput").ap()
            d_kt = nc.dram_tensor("d_kt", [128, MT, S], BF16, kind="ExternalOutput").ap()
            d_v = nc.dram_tensor("d_v", [128, ST, NHL, HD + 1], BF16, kind="ExternalOutput").ap()
            d_ot = nc.dram_tensor("d_ot", [128, MT, S], BF16, kind="ExternalOutput").ap()
            nc.sync.dma_start(d_qt, QT_sb[:])
            nc.sync.dma_start(d_kt, KT_sb[:])
            nc.sync.dma_start(d_v, V_sb[:])
            nc.sync.dma_start(d_ot, oT_sb[:])

        # ---- out-projection tail ----
        for qt in range(8, ST):
            outproj_group(qt)


_CACHED = {}


def _get_bass():
    if "nc" not in _CACHED:
        _CACHED["nc"] = _build_bass()
    return _CACHED["nc"]


def _prep_core_inputs(c, query, key, value, Wq, bq, Wk, bk, Wv, bv, Wo):
    b, half = c // 2, c % 2
    sl = slice(DLOC * half, DLOC * half + DLOC)
    bq_sl = (bq[sl] * 0.125).astype(np.float32).reshape(MT, 128).T.copy()
    bk_sl = bk[sl].astype(np.float32).reshape(MT, 128).T.copy()
    return {
        "xqT": np.ascontiguousarray(query[b].T).astype(NPBF),
        "xkT": np.ascontiguousarray(key[b].T).astype(NPBF),
        "xvT": np.ascontiguousarray(value[b].T).astype(NPBF),
        "wq": np.ascontiguousarray(Wq[sl, :].T * 0.125).astype(NPBF),
        "wk": np.ascontiguousarray(Wk[sl, :].T).astype(NPBF),
        "wv": np.ascontiguousarray(Wv[sl, :].T).astype(NPBF),
        "wo": np.ascontiguousarray(Wo[:, sl].T).astype(NPBF),
        "bq": np.ascontiguousarray(bq_sl),
        "bk": np.ascontiguousarray(bk_sl),
        "bv": bv[sl].astype(np.float32).reshape(1, DLOC).copy(),
    }


def kernel(query, key, value, Wq, bq, Wk, bk, Wv, bv, Wo, bo,
           trace=False, **run_kwargs):
    query = np.asarray(query, np.float32)
    key = np.asarray(key, np.float32)
    value = np.asarray(value, np.float32)
    Wq, Wk, Wv, Wo = (np.asarray(w, np.float32) for w in (Wq, Wk, Wv, Wo))
    bq, bk, bv, bo = (np.asarray(x, np.float32) for x in (bq, bk, bv, bo))

    nc = _get_bass()
    in_maps = [_prep_core_inputs(c, query, key, value, Wq, bq, Wk, bk, Wv, bv, Wo)
               for c in range(8)]
    res = run_bass_kernel_spmd(nc, in_maps, core_ids=list(range(8)),
                               trace=trace, **run_kwargs)
    _CACHED["last_result"] = res

    B = query.shape[0]
    out = np.empty((B, S, E), np.float32)
    for b in range(B):
        out[b] = (res.results[2 * b]["out"].astype(np.float32)
                  + res.results[2 * b + 1]["out"].astype(np.float32) + bo)
    return out
